# revision 35
# baseline (speedup 1.0000x reference)
"""Trainium2 Bass kernel for a 1-layer transformer encoder.

Reference model (B=32, S=512, D=768, H=12, hd=64, hidden=3072):
    q,k,v = x@Wq, x@Wk, x@Wv         (per head)
    attn  = softmax(q k^T / 8) v
    mha   = concat_heads @ Wo
    out1  = x + LN(mha)
    ffn   = gelu(out1@W1 + b1) @ W2 + b2
    out   = out1 + LN(ffn)

Sharding: data-parallel over batch -- each of the 8 cores gets 4 full
sequences (2048 tokens) and all weights; no collectives.

v2 structure (vs the 637us baseline):
  * x is shipped twice from host: fp32 [T,D] for the residual and
    pre-transposed bf16 xT [D,T] for all matmuls -- kills the on-device
    cast + 96 PE transposes and the input-DMA ramp stall.
  * Per-sequence merged segments: attn@v -> Wo -> LN1 -> out1 transpose
    for sequence g while the *next* sequence's score matmuls + exps run
    ahead (pts pool bufs=1 provides the handoff), keeping PE dense and
    ACT's exp stream off the critical path.
  * Score matmuls (K=64) row-packed two heads at a time via
    tile_position row groups -> 2x effective PE rate.
  * Softmax denominator: ones-column in v (row 64 of the av output);
    reciprocal_approx_fast directly on the PSUM row, gpsimd
    partition_broadcast instead of a DRAM bounce.
  * LN rstd: variances batched -> one Sqrt call per pair of chunks so
    the exp<->sqrt ACT table switches stay ~2 per segment.
  * o1T (FFN input layout) spilled to DRAM and re-read per sequence in
    the FFN phase to stay inside SBUF.
"""

import os
import sys

import numpy as np

for _p in ("/opt/trn_rl_repo", "/root/.axon_site/_ro/trn_rl_repo"):
    if os.path.isdir(_p) and _p not in sys.path:
        sys.path.insert(0, _p)

os.environ.setdefault("MYCRO_LOCAL_CACHE", "1")

import concourse.bacc as bacc
import concourse.tile as tile
from concourse import mybir
from concourse.bass_utils import run_bass_kernel_spmd
from concourse.masks import make_identity

F32 = mybir.dt.float32
BF16 = mybir.dt.bfloat16
AF = mybir.ActivationFunctionType
OP = mybir.AluOpType

# model dims
D, NHEAD, HD, FF, SEQ, P = 768, 12, 64, 3072, 512, 128
ND = D // P    # 6 feature chunks
NF = FF // P   # 24 hidden chunks
NHP = NHEAD // 2  # 6 head pairs
EPS = 1e-5
N_CORES = 8
B_TOTAL = 32


def emit(nc, tc, io, bpc, trivial_ln):
    T = bpc * SEQ
    NT = T // P          # 16 token chunks
    NB = bpc             # sequences per core

    consts = tc.alloc_tile_pool(name="consts", bufs=1)
    psp = tc.alloc_tile_pool(name="psp", bufs=1, space="PSUM")
    dramp = tc.alloc_tile_pool(name="dramp", bufs=1, space="DRAM")

    # ---- constants ----
    identf = consts.tile([P, P], F32)
    make_identity(nc, identf)
    ones1 = consts.tile([1, P], BF16)
    nc.vector.memset(ones1, 1.0)
    eps_t = consts.tile([P, 1], F32)
    nc.vector.memset(eps_t, EPS)
    b2r = consts.tile([1, D], BF16)
    nc.sync.dma_start(b2r, io["b2r"])
    b1t = consts.tile([P, NF], F32)
    nc.sync.dma_start(b1t, io["b1t"])
    ln1w = ln1b = None
    if not trivial_ln:
        ln1w = consts.tile([P, D], BF16, tag="ln1w", name="ln1w_bc")
        nc.gpsimd.dma_start(ln1w, io["ln1w"].broadcast_to([P, D]))
        ln1b = consts.tile([P, D], BF16, tag="ln1b", name="ln1b_bc")
        nc.gpsimd.dma_start(ln1b, io["ln1b"].broadcast_to([P, D]))

    spill = dramp.tile([T, D], F32, tag="spill", name="out1_spill")
    o1t_d = dramp.tile([D, T], BF16, tag="o1t_d", name="o1t_spill")

    # ---- right stack: xT + qkv weights (phase A only) ----
    xtp = tc.alloc_tile_pool(name="xtp", bufs=1, side="right")
    xt = [xtp.tile([P, T], BF16, tag=f"xt{k}", name=f"xt{k}") for k in range(ND)]
    wqkp = tc.alloc_tile_pool(name="wqkp", bufs=1, side="right")
    wvp_pool = tc.alloc_tile_pool(name="wvp", bufs=1, side="right")
    wv_sb, wq_sb, wk_sb = [], [], []
    for k in range(ND):
        t = wvp_pool.tile([P, D], BF16, tag=f"wv{k}", name=f"wv{k}_sb")
        nc.sync.dma_start(t, io["wv"][k * P:(k + 1) * P, :])
        wv_sb.append(t)
    # xT arrives in [128, 512] blocks, sequence-major so V can start early
    for g in range(NB):
        for k in range(ND):
            nc.sync.dma_start(xt[k][:, g * SEQ:(g + 1) * SEQ],
                              io["xT"][k * P:(k + 1) * P, g * SEQ:(g + 1) * SEQ])
    for lst, name in ((wq_sb, "wq"), (wk_sb, "wk")):
        for k in range(ND):
            t = wqkp.tile([P, D], BF16, tag=f"{name}{k}", name=f"{name}{k}_sb")
            nc.sync.dma_start(t, io[name][k * P:(k + 1) * P, :])
            lst.append(t)
    wop = tc.alloc_tile_pool(name="wop", bufs=1)
    wo_sb = []
    for k in range(ND):
        t = wop.tile([P, D], BF16, tag=f"wo{k}", name=f"wo{k}_sb")
        nc.sync.dma_start(t, io["wo"][k * P:(k + 1) * P, :])
        wo_sb.append(t)

    # ---- long-lived left pools ----
    qkT = tc.alloc_tile_pool(name="qkT", bufs=1)
    qT = [qkT.tile([P, T], BF16, tag=f"qT{m}", name=f"qT{m}") for m in range(ND)]
    kT = [qkT.tile([P, T], BF16, tag=f"kT{m}", name=f"kT{m}") for m in range(ND)]
    vp = tc.alloc_tile_pool(name="vp", bufs=1)
    VW = NHEAD * (HD + 1)
    vN = [vp.tile([P, VW], BF16, tag=f"v{i}", name=f"v{i}") for i in range(NT)]

    # rotating pools for the segment loop; ptp is top-of-stack so it can
    # release before the W1 prefetch in the last segment
    otp = tc.alloc_tile_pool(name="otp", bufs=2)      # oT per-seq + den helpers
    lnp = tc.alloc_tile_pool(name="lnp", bufs=2)      # LN1 working tiles
    ptp = tc.alloc_tile_pool(name="ptp", bufs=1)      # exp'd scores (bufs=1 handoff)

    # ---- phase 1: V projection (stationary = xT token chunk) ----
    for i in range(NT):
        psv = psp.tile([P, 2 * SEQ], F32, tag="ps2", bufs=2, name="psv")[:, :D]
        for k in range(ND):
            lhs = xt[k][:, i * P:(i + 1) * P]
            nc.tensor.matmul(psv[:, :SEQ], lhs, wv_sb[k][:, :SEQ],
                             start=(k == 0), stop=(k == ND - 1))
            nc.tensor.matmul(psv[:, SEQ:D], lhs, wv_sb[k][:, SEQ:D],
                             start=(k == 0), stop=(k == ND - 1))
        nc.vector.memset(vN[i][:, HD::HD + 1], 1.0)
        nc.any.tensor_copy(
            out=vN[i].rearrange("p (h w) -> p h w", w=HD + 1)[:, :, 0:HD],
            in_=psv.rearrange("p (h w) -> p h w", w=HD))
    wvp_pool.release()

    pts = {}  # (g%1? keyed (hp, h, pair)) -> tile; bufs=1 rotation is the pipeline handoff

    def emit_scores(g, hp):
        """Scores + exp for sequence g, head pair hp (row-packed K=64)."""
        for pair in range(2):
            sts = [psp.tile([P, 2 * SEQ], F32, tag="ps2", bufs=2, name="st")
                   for _ in range(2)]
            # interleave the two heads' matmuls so the row groups overlap
            for t2 in range(2):
                kc = pair * 2 + t2
                for h in range(2):
                    p0 = h * HD
                    nc.tensor.matmul(
                        sts[h][:, t2 * SEQ:(t2 + 1) * SEQ],
                        kT[hp][p0:p0 + HD,
                               g * SEQ + kc * P: g * SEQ + (kc + 1) * P],
                        qT[hp][p0:p0 + HD, g * SEQ:(g + 1) * SEQ],
                        start=True, stop=True)
            for h in range(2):
                pt = ptp.tile([P, 2 * SEQ], BF16, tag=f"pt{hp}_{h}_{pair}",
                              bufs=1, name=f"pt{hp}_{h}_{pair}")
                nc.scalar.activation(out=pt, in_=sts[h], func=AF.Exp)
                pts[(hp, h, pair)] = pt

    # ---- phase 2: Q/K projections + scores for sequence 0 ----
    for m in range(ND):
        for dst, w_sb in ((qT, wq_sb), (kT, wk_sb)):
            pss = [psp.tile([P, SEQ], F32, tag="ps1", bufs=4, name=f"qk{g}")
                   for g in range(NB)]
            for k in range(ND):
                lhs = w_sb[k][:, m * P:(m + 1) * P]
                for g in range(NB):
                    nc.tensor.matmul(
                        pss[g], lhs, xt[k][:, g * SEQ:(g + 1) * SEQ],
                        start=(k == 0), stop=(k == ND - 1))
            for g in range(NB):
                nc.any.tensor_copy(
                    out=dst[m][:, g * SEQ:(g + 1) * SEQ], in_=pss[g])
        emit_scores(0, m)

    # xT and the qkv weights are dead after QK; free the right stack now
    wqkp.release()
    xtp.release()

    # ---- segments: per sequence g: av, (scores g+1), Wo, LN1, transpose ----
    w1p = None
    w1_sb = []
    x_res = {}

    for g in range(NB):
        oTg = [otp.tile([P, SEQ], BF16, tag=f"oT{kk}", bufs=1, name=f"oT{kk}")
               for kk in range(ND)]
        for hp in range(NHP):
            # av for both heads of hp (stationary = v cols + ones col)
            for h in range(2):
                head = hp * 2 + h
                ot = psp.tile([HD + 1, SEQ], F32, tag="ps1", bufs=4, name="ot")
                for kc in range(4):
                    vblk = vN[g * 4 + kc][:, head * (HD + 1):(head + 1) * (HD + 1)]
                    nc.tensor.matmul(
                        ot, vblk,
                        pts[(hp, h, kc // 2)][:, (kc % 2) * SEQ:(kc % 2 + 1) * SEQ],
                        start=(kc == 0), stop=(kc == 3))
                # denominator -> reciprocal -> broadcast -> scale
                # (copy the PSUM row to partition 0 first: custom-DVE ops
                # can't take the partition-shifted PSUM read directly)
                dstg = otp.tile([1, SEQ], F32, tag="rd", bufs=2, name="dstg")
                nc.vector.tensor_copy(out=dstg, in_=ot[HD:HD + 1, :])
                rd = otp.tile([1, SEQ], F32, tag="rd", bufs=2, name="rd")
                nc.vector.reciprocal_approx_fast(out=rd, in_=dstg)
                rdb = otp.tile([P, SEQ], F32, tag="rdb", bufs=1, name="rdb")
                nc.gpsimd.partition_broadcast(rdb, rd)
                if h == 0:
                    nc.vector.tensor_mul(out=oTg[hp][0:HD, :],
                                         in0=ot[0:HD, :], in1=rdb[0:HD, :])
                else:
                    # align partitions: copy-shift first, then scale in place
                    nc.vector.tensor_copy(out=oTg[hp][HD:P, :], in_=ot[0:HD, :])
                    nc.vector.tensor_mul(out=oTg[hp][HD:P, :],
                                         in0=oTg[hp][HD:P, :], in1=rdb[HD:P, :])
            if g + 1 < NB:
                emit_scores(g + 1, hp)

        if g == NB - 1:
            # last av consumed pts; free it and prefetch W1 into the space
            ptp.release()
            w1p = tc.alloc_tile_pool(name="w1p", bufs=1, side="right")
            w1_sb = [w1p.tile([P, FF], BF16, tag=f"w1_{k}", name=f"w1_{k}")
                     for k in range(ND)]
            for k in range(ND):
                nc.sync.dma_start(w1_sb[k], io["w1"][k * P:(k + 1) * P, :])

        # Wo + LN1 in chunk pairs (one Sqrt per pair keeps table switches low)
        for cp in range(2):
            for c2 in range(2):
                c = cp * 2 + c2
                i = g * 4 + c
                x_t = lnp.tile([P, D], BF16, tag="xres", bufs=2, name="x_t")
                nc.sync.dma_start(x_t, io["xr"][i * P:(i + 1) * P, :])
                x_res[c] = x_t
            mhs, mvs = [], []
            for c2 in range(2):
                c = cp * 2 + c2
                mh = psp.tile([P, 2 * SEQ], F32, tag="ps2", bufs=2,
                              name="mh")[:, :D]
                for k in range(ND):
                    lhs = oTg[k][:, c * P:(c + 1) * P]
                    nc.tensor.matmul(mh[:, :SEQ], lhs, wo_sb[k][:, :SEQ],
                                     start=(k == 0), stop=(k == ND - 1))
                    nc.tensor.matmul(mh[:, SEQ:D], lhs, wo_sb[k][:, SEQ:D],
                                     start=(k == 0), stop=(k == ND - 1))
                stats = lnp.tile([P, 3, 6], F32, tag="stats", bufs=2, name="stats")
                for sg in range(3):
                    nc.vector.bn_stats(out=stats[:, sg, :],
                                       in_=mh[:, sg * 256:(sg + 1) * 256])
                mv = lnp.tile([P, 2], F32, tag=f"mv{c2}", bufs=2, name="mv")
                nc.vector.bn_aggr(out=mv, in_=stats)
                mhs.append(mh)
                mvs.append(mv)
            var2 = lnp.tile([P, 2], F32, tag="var2", bufs=2, name="var2")
            for c2 in range(2):
                nc.vector.tensor_copy(out=var2[:, c2:c2 + 1], in_=mvs[c2][:, 1:2])
            std2 = lnp.tile([P, 2], F32, tag="std2", bufs=2, name="std2")
            nc.scalar.activation(out=std2, in_=var2, func=AF.Sqrt,
                                 bias=eps_t, scale=1.0)
            rstd2 = lnp.tile([P, 2], F32, tag="rstd2", bufs=2, name="rstd2")
            nc.vector.reciprocal_approx_fast(out=rstd2, in_=std2)
            for c2 in range(2):
                c = cp * 2 + c2
                i = g * 4 + c
                if trivial_ln:
                    res_in = x_res[c]
                    u = lnp.tile([P, D], F32, tag="u", bufs=2, name="u")
                    nc.vector.tensor_scalar(
                        out=u, in0=mhs[c2], scalar1=mvs[c2][:, 0:1],
                        scalar2=None, op0=OP.subtract)
                else:
                    res_in = lnp.tile([P, D], F32, tag="xb", bufs=1, name="xb")
                    nc.gpsimd.tensor_add(out=res_in, in0=x_res[c], in1=ln1b)
                    u = lnp.tile([P, D], F32, tag="u", bufs=2, name="u")
                    nc.vector.scalar_tensor_tensor(
                        out=u, in0=mhs[c2], scalar=mvs[c2][:, 0:1], in1=ln1w,
                        op0=OP.subtract, op1=OP.mult)
                out1_t = lnp.tile([P, D], F32, tag="u", bufs=2, name="out1_t")
                nc.vector.scalar_tensor_tensor(
                    out=out1_t, in0=u, scalar=rstd2[:, c2:c2 + 1], in1=res_in,
                    op0=OP.mult, op1=OP.add)
                nc.sync.dma_start(spill[i * P:(i + 1) * P, :], out1_t)
                o1s = lnp.tile([P, D], BF16, tag="o1s", bufs=1, name="o1s")
                for j in range(ND):
                    ptr = psp.tile([P, P], F32, tag="ps1", bufs=4, name="ptr")
                    nc.tensor.transpose(ptr, out1_t[:, j * P:(j + 1) * P], identf)
                    nc.any.tensor_copy(out=o1s[:, j * P:(j + 1) * P], in_=ptr)
                nc.sync.dma_start(
                    o1t_d.rearrange("(j p) t -> p j t", p=P)[:, :, i * P:(i + 1) * P],
                    o1s.rearrange("p (j q) -> p j q", q=P))

    lnp.release()
    otp.release()
    vp.release()
    qkT.release()
    wop.release()

    # ---- phase B: FFN + LN2 ----
    w2p = tc.alloc_tile_pool(name="w2p", bufs=1, side="right")
    w2_sb = [w2p.tile([P, D], BF16, tag=f"w2_{k}", name=f"w2_{k}") for k in range(NF)]
    for k in range(NF):
        nc.sync.dma_start(w2_sb[k], io["w2"][k * P:(k + 1) * P, :])

    trC = tc.alloc_tile_pool(name="trC", bufs=2)
    ln2w = ln2b = None
    if not trivial_ln:
        ln2w = trC.tile([P, D], BF16, tag="ln2w", bufs=1, name="ln2w_bc")
        nc.gpsimd.dma_start(ln2w, io["ln2w"].broadcast_to([P, D]))
        ln2b = trC.tile([P, D], BF16, tag="ln2b", bufs=1, name="ln2b_bc")
        nc.gpsimd.dma_start(ln2b, io["ln2b"].broadcast_to([P, D]))
    hbuf = tc.alloc_tile_pool(name="hbuf", bufs=1)
    o1p = tc.alloc_tile_pool(name="o1p", bufs=2)

    o1T = {}
    def prefetch_o1T(g):
        for k in range(ND):
            t = o1p.tile([P, SEQ], BF16, tag=f"o1T{k}", name=f"o1T{k}")
            nc.sync.dma_start(t, o1t_d[k * P:(k + 1) * P, g * SEQ:(g + 1) * SEQ])
            o1T[(g % 2, k)] = t

    prefetch_o1T(0)
    for g in range(NB):
        if g + 1 < NB:
            prefetch_o1T(g + 1)
        o1Tg = [o1T[(g % 2, k)] for k in range(ND)]
        hts = []
        for f in range(NF):
            hp_ps = psp.tile([P, SEQ], F32, tag="ps1", bufs=4, name="hp")
            for k in range(ND):
                nc.tensor.matmul(
                    hp_ps, w1_sb[k][:, f * P:(f + 1) * P], o1Tg[k],
                    start=(k == 0), stop=(k == ND - 1))
            ht = hbuf.tile([P, SEQ], BF16, tag=f"ht{f}", name=f"ht{f}")
            nc.scalar.activation(out=ht, in_=hp_ps, func=AF.Gelu,
                                 bias=b1t[:, f:f + 1], scale=1.0)
            hts.append(ht)
        for cp in range(2):
            fps, mvs, o1ins = [], [], []
            for c2 in range(2):
                sc = cp * 2 + c2
                i = g * 4 + sc
                fp = psp.tile([P, 2 * SEQ], F32, tag="ps2", bufs=2,
                              name="fp")[:, :D]
                for f in range(NF):
                    lhs = hts[f][:, sc * P:(sc + 1) * P]
                    nc.tensor.matmul(fp[:, :SEQ], lhs, w2_sb[f][:, :SEQ],
                                     start=(f == 0), stop=False)
                    nc.tensor.matmul(fp[:, SEQ:D], lhs, w2_sb[f][:, SEQ:D],
                                     start=(f == 0), stop=False)
                nc.tensor.matmul(fp[:, :SEQ], ones1, b2r[:, :SEQ],
                                 start=False, stop=True)
                nc.tensor.matmul(fp[:, SEQ:D], ones1, b2r[:, SEQ:D],
                                 start=False, stop=True)
                o1in = trC.tile([P, D], F32, tag="o1in", bufs=3, name="o1in")
                nc.sync.dma_start(o1in, spill[i * P:(i + 1) * P, :])
                stats = trC.tile([P, 3, 6], F32, tag="stats2", bufs=2, name="stats2")
                for sg in range(3):
                    nc.vector.bn_stats(out=stats[:, sg, :],
                                       in_=fp[:, sg * 256:(sg + 1) * 256])
                mv = trC.tile([P, 2], F32, tag=f"mv2_{c2}", bufs=2, name="mv2")
                nc.vector.bn_aggr(out=mv, in_=stats)
                fps.append(fp)
                mvs.append(mv)
                o1ins.append(o1in)
            var2 = trC.tile([P, 2], F32, tag="var2b", bufs=2, name="var2b")
            for c2 in range(2):
                nc.vector.tensor_copy(out=var2[:, c2:c2 + 1], in_=mvs[c2][:, 1:2])
            std2 = trC.tile([P, 2], F32, tag="std2b", bufs=2, name="std2b")
            nc.scalar.activation(out=std2, in_=var2, func=AF.Sqrt,
                                 bias=eps_t, scale=1.0)
            rstd2 = trC.tile([P, 2], F32, tag="rstd2b", bufs=2, name="rstd2b")
            nc.vector.reciprocal_approx_fast(out=rstd2, in_=std2)
            for c2 in range(2):
                sc = cp * 2 + c2
                i = g * 4 + sc
                if trivial_ln:
                    base = o1ins[c2]
                    u = trC.tile([P, D], F32, tag="u2", bufs=2, name="u2")
                    nc.vector.tensor_scalar(
                        out=u, in0=fps[c2], scalar1=mvs[c2][:, 0:1],
                        scalar2=None, op0=OP.subtract)
                else:
                    base = trC.tile([P, D], F32, tag="base", bufs=2, name="base")
                    nc.gpsimd.tensor_add(out=base, in0=o1ins[c2], in1=ln2b)
                    u = trC.tile([P, D], F32, tag="u2", bufs=2, name="u2")
                    nc.vector.scalar_tensor_tensor(
                        out=u, in0=fps[c2], scalar=mvs[c2][:, 0:1], in1=ln2w,
                        op0=OP.subtract, op1=OP.mult)
                outt = trC.tile([P, D], F32, tag="outt", bufs=3, name="outt")
                nc.vector.scalar_tensor_tensor(
                    out=outt, in0=u, scalar=rstd2[:, c2:c2 + 1], in1=base,
                    op0=OP.mult, op1=OP.add)
                nc.sync.dma_start(io["out"][i * P:(i + 1) * P, :], outt)

    o1p.release()
    hbuf.release()
    trC.release()
    w2p.release()
    if w1p is not None:
        w1p.release()
    consts.release()
    psp.release()
    dramp.release()


def build(bpc, trivial_ln=True):
    """Build + compile the per-core program. Returns the Bacc object."""
    T = bpc * SEQ
    nc = bacc.Bacc("TRN2", target_bir_lowering=False, debug=False,
                   num_devices=N_CORES)
    io = {
        "xr": nc.dram_tensor("xr", [T, D], BF16, kind="ExternalInput").ap(),
        "xT": nc.dram_tensor("xT", [D, T], BF16, kind="ExternalInput").ap(),
        "wq": nc.dram_tensor("wq", [D, D], BF16, kind="ExternalInput").ap(),
        "wk": nc.dram_tensor("wk", [D, D], BF16, kind="ExternalInput").ap(),
        "wv": nc.dram_tensor("wv", [D, D], BF16, kind="ExternalInput").ap(),
        "wo": nc.dram_tensor("wo", [D, D], BF16, kind="ExternalInput").ap(),
        "w1": nc.dram_tensor("w1", [D, FF], BF16, kind="ExternalInput").ap(),
        "w2": nc.dram_tensor("w2", [FF, D], BF16, kind="ExternalInput").ap(),
        "b1t": nc.dram_tensor("b1t", [P, NF], F32, kind="ExternalInput").ap(),
        "b2r": nc.dram_tensor("b2r", [1, D], BF16, kind="ExternalInput").ap(),
        "ln1w": nc.dram_tensor("ln1w", [1, D], F32, kind="ExternalInput").ap(),
        "ln1b": nc.dram_tensor("ln1b", [1, D], F32, kind="ExternalInput").ap(),
        "ln2w": nc.dram_tensor("ln2w", [1, D], F32, kind="ExternalInput").ap(),
        "ln2b": nc.dram_tensor("ln2b", [1, D], F32, kind="ExternalInput").ap(),
        "out": nc.dram_tensor("out", [T, D], F32, kind="ExternalOutput").ap(),
    }
    with tile.TileContext(nc) as tc:
        emit(nc, tc, io, bpc, trivial_ln)
    nc.compile()
    return nc


def _ln_trivial(inputs):
    return (np.all(np.asarray(inputs["ln1_w"]) == 1.0)
            and np.all(np.asarray(inputs["ln2_w"]) == 1.0)
            and np.all(np.asarray(inputs["ln1_b"]) == 0.0)
            and np.all(np.asarray(inputs["ln2_b"]) == 0.0))


def prep_weights(inputs):
    """Host-side weight layout prep (numpy only)."""
    bf = mybir.dt.np(BF16)
    f32 = np.float32
    wq = (np.asarray(inputs["Wq"], f32).transpose(1, 0, 2).reshape(D, D)
          / np.sqrt(HD)).astype(bf)
    wk = np.asarray(inputs["Wk"], f32).transpose(1, 0, 2).reshape(D, D).astype(bf)
    wv = np.asarray(inputs["Wv"], f32).transpose(1, 0, 2).reshape(D, D).astype(bf)
    return {
        "wq": np.ascontiguousarray(wq),
        "wk": np.ascontiguousarray(wk),
        "wv": np.ascontiguousarray(wv),
        "wo": np.asarray(inputs["Wo"], f32).astype(bf),
        "w1": np.asarray(inputs["W1"], f32).astype(bf),
        "w2": np.asarray(inputs["W2"], f32).astype(bf),
        "b1t": np.ascontiguousarray(
            np.asarray(inputs["b1"], f32).reshape(NF, P).T),
        "b2r": np.asarray(inputs["b2"], f32).reshape(1, D).astype(bf),
        "ln1w": np.asarray(inputs["ln1_w"], f32).reshape(1, D),
        "ln1b": np.asarray(inputs["ln1_b"], f32).reshape(1, D),
        "ln2w": np.asarray(inputs["ln2_w"], f32).reshape(1, D),
        "ln2b": np.asarray(inputs["ln2_b"], f32).reshape(1, D),
    }


def make_in_maps(inputs):
    """Per-core input shards: x fp32 [T,D] + transposed bf16 xT [D,T]."""
    bpc = B_TOTAL // N_CORES
    w = prep_weights(inputs)
    bf = mybir.dt.np(BF16)
    x = np.asarray(inputs["x"], np.float32)
    in_maps = []
    for c in range(N_CORES):
        shard = np.ascontiguousarray(
            x[c * bpc:(c + 1) * bpc].reshape(bpc * SEQ, D))
        xT = np.ascontiguousarray(shard.T.astype(bf))
        in_maps.append({"xr": shard.astype(bf), "xT": xT, **w})
    return in_maps


_cache = {}


def kernel(**inputs) -> np.ndarray:
    bpc = B_TOTAL // N_CORES
    key = ("nc", _ln_trivial(inputs))
    if key not in _cache:
        _cache[key] = build(bpc, trivial_ln=key[1])
    _cache["nc"] = nc = _cache[key]
    in_maps = make_in_maps(inputs)
    res = run_bass_kernel_spmd(nc, in_maps, list(range(N_CORES)))
    out = np.concatenate(
        [res.results[c]["out"].reshape(bpc, SEQ, D) for c in range(N_CORES)],
        axis=0)
    return np.ascontiguousarray(out.astype(np.float32))
